# revision 6
# baseline (speedup 1.0000x reference)
"""Trainium2 Bass kernel for nn_ButterflyFilter.

The reference applies, per length-512 row (flattened b*c*angles):
  zero-pad to 1024 -> 10-stage butterfly "FFT" (stage order decreasing)
  -> elementwise filter (bit-reversed order) -> 10-stage butterfly
  "IFFT" (stage order increasing) -> real part of first 512 entries.

Every step is linear in x, so the whole chain is one complex 1024x1024
operator A determined by (twiddle_fft, twiddle_ifft, fourier_filter_br).
Since x is real with support on [:512] and only Re(y)[:512] is kept, the
effective map is the real 512x512 matrix W = Re(A)[:512, :512]:

    proj_row = W @ x_row

For the actual FBP ramp-filter parameters, W is a circular-convolution
(Toeplitz) matrix whose kernel decays as 1/k^2: truncating it to a band
of half-width 64 costs ~1.7e-4 relative error (gate is 2e-2). Each
128-row output chunk b then only needs contraction rows
[128b-64, 128b+192), i.e. TWO 128-row windows from the offset-64
partitioning of the input rows -- 2 matmuls per chunk instead of 4, in
bf16 (~2.6e-3 total err). The device work per core is 2 (b,c) tiles x
4 output chunks x 2 matmuls. If the runtime-composed W turns out not to
be banded (e.g. random twiddles), we fall back to the exact dense fp32r
path automatically.
"""

import os
import sys
import types
from contextlib import ExitStack

import numpy as np

import concourse.bass as bass
import concourse.mybir as mybir
from concourse.bass_utils import run_bass_kernel_spmd


def _ensure_axon_hooks():
    # concourse.bass_utils imports antenv.axon_hooks on the trace path; some
    # images lack that module. Provide a no-op holder so a BASS_TRACE env set
    # by the caller can't crash the run.
    try:
        import antenv.axon_hooks  # noqa: F401
    except Exception:
        m = types.ModuleType("antenv.axon_hooks")
        m._h = None
        m.set_axon_ntff_profile_hook = lambda h: setattr(m, "_h", h)
        m.get_axon_ntff_profile_hook = lambda: m._h
        sys.modules["antenv.axon_hooks"] = m


_ensure_axon_hooks()

N_CORES = 8
S = 512          # input/output row length
NF = 1024        # padded length
P = 128          # SBUF partitions
BC_PER_CORE = 2  # 16 (b,c) tiles / 8 cores

BAND_ERR_MAX = 5e-3  # operator-level truncation error gate for band path

# ---------------------------------------------------------------------------
# Band scheme geometry.
#
# Input rows are partitioned at the offset-64 boundaries [0,64,192,320,448,
# 512]; chunks c0=[0,64) and c4=[448,512) share one 128-partition SBUF block
# (c0 in partitions 0:64, c4 in 64:128), giving 4 x-blocks per (b,c) tile.
# Output chunk b consumes x-blocks {b, b+1 mod wrap} via W column-tiles:
#   wcol0 = b0j0 (c0 rows, partitions 0:64) + b3j1 (c4 rows, partitions
#   64:128); wcol1..6 = the full-128-row tiles.
# Per output chunk b: [(wcol, partition range, x block)] x 2 matmuls.
BAND_PLAN = {
    0: [(0, (0, 64), 0), (1, (0, 128), 1)],
    1: [(2, (0, 128), 1), (3, (0, 128), 2)],
    2: [(4, (0, 128), 2), (5, (0, 128), 3)],
    3: [(6, (0, 128), 3), (0, (64, 128), 0)],
}

# The input is streamed over BOTH HWDGE queues in parallel (SP=sync and
# Activation=scalar engines each own one hardware queue). DRAM slabs are
# (128, cols) bf16, partition-major, split into contiguous column pieces;
# each piece's completion bumps that queue's semaphore by 16, and in-queue
# completion order lets one cumulative threshold per queue gate each matmul
# group.
#
# slab_p (sync queue), column layout and pieces:
#   P1 [0,768):     wcol0 | wcol1 | x0 blk0          -> with Q1: bc0 b0
#   P2 [768,1536):  wcol2 | wcol3 | x0 blk2          -> bc0 b1 (+Q1)
#   P3 [1536,1664): wcol6                            -> bc0 b3 (+Q2)
#   P4 [1664,2176): x1 blk2                          -> bc1 b1
# slab_q (scalar queue):
#   Q1 [0,512):     x0 blk1                          -> bc0 b0/b1
#   Q2 [512,1280):  wcol4 | wcol5 | x0 blk3          -> bc0 b2/b3
#   Q3 [1280,2304): x1 blk0 | x1 blk1               -> bc1 b0
#   Q4 [2304,2816): x1 blk3                          -> bc1 b2/b3
P_COLS = 2176
Q_COLS = 2816
P_PIECES = [(0, 768), (768, 1536), (1536, 1664), (1664, 2176)]
Q_PIECES = [(0, 512), (512, 1280), (1280, 2304), (2304, 2816)]
# Per (bc, b) group: required (sync_pieces, scalar_pieces) counts.
GROUP_THRESH = {
    (0, 0): (1, 1), (0, 1): (2, 1), (0, 2): (2, 2), (0, 3): (3, 2),
    (1, 0): (3, 3), (1, 1): (4, 3), (1, 2): (4, 4), (1, 3): (4, 4),
}
# SBUF addresses: wcol index -> (slab, col); (bc, blk) -> (slab, col).
WCOL_AT = {0: ("p", 0), 1: ("p", 128), 2: ("p", 768), 3: ("p", 896),
           4: ("q", 512), 5: ("q", 640), 6: ("p", 1536)}
XBLK_AT = {(0, 0): ("p", 256), (0, 1): ("q", 0), (0, 2): ("p", 1024),
           (0, 3): ("q", 768), (1, 0): ("q", 1280), (1, 1): ("q", 1792),
           (1, 2): ("p", 1664), (1, 3): ("q", 2304)}

# Exposed for the test harness: exec time of the last device run (ns), if
# profiling was enabled via BUTTERFLY_TRACE=1.
last_exec_time_ns = None
last_results = None
last_path = None  # "band" or "dense", for the harness/debugging


def _bf16():
    return mybir.dt.np(mybir.dt.bfloat16)


def _butterfly_np(tw, x, increasing):
    # Mirrors the reference butterfly exactly, in numpy (any dtype).
    B, n = x.shape
    m = tw.shape[0]
    order = range(m) if increasing else range(m - 1, -1, -1)
    for idx in order:
        s = 1 << idx
        t = tw[idx].reshape(n // (2 * s), s, 2, 2)
        xr = x.reshape(B, n // (2 * s), 2, s)
        x = np.einsum('gjik,bgkj->bgij', t, xr).reshape(B, n)
    return x


def _compose_wt(twiddle_fft, twiddle_ifft, fourier_filter_br):
    """Fold twiddles+filter into the lhsT operand Wt[i_in, o_out] (512x512 f32)."""
    tw_fft = np.asarray(twiddle_fft, dtype=np.float64)
    tw_ifft = np.asarray(twiddle_ifft, dtype=np.float64)
    filt = np.asarray(fourier_filter_br, dtype=np.float64)
    tf = tw_fft[0, ..., 0] + 1j * tw_fft[0, ..., 1]
    ti = tw_ifft[0, ..., 0] + 1j * tw_ifft[0, ..., 1]
    X = np.eye(NF, dtype=np.complex128)      # row j = e_j
    X = _butterfly_np(tf, X, increasing=False)
    X = X * filt[None, :]
    X = _butterfly_np(ti, X, increasing=True)
    # X = chain(I) = A^T, so X[i, o] = A[o, i]; W[o, i] = Re(A[o, i]).
    # lhsT for out = lhsT.T @ rhs must be Wt[i, o] = W[o, i] = Re(X[i, o]).
    return np.ascontiguousarray(np.real(X[:S, :S]).astype(np.float32))


def _band_error(wt):
    """Relative Frobenius mass of W outside the 2-window band cover."""
    cov = np.zeros((S, S), bool)  # indexed [i, o] like wt
    for b in range(4):
        lo, hi = max(0, 128 * b - 64), min(S, 128 * b + 192)
        cov[lo:hi, 128 * b:128 * b + 128] = True
    tot = float(np.square(wt).sum())
    off = float(np.square(wt[~cov]).sum())
    return (off / tot) ** 0.5 if tot > 0 else 0.0


def _band_wcols(wt):
    wc = np.zeros((7, P, P), np.float32)
    wc[0][0:64] = wt[0:64, 0:128]          # b0 j0 (chunk c0)
    wc[0][64:128] = wt[448:512, 384:512]   # b3 j1 (chunk c4)
    wc[1] = wt[64:192, 0:128]    # b0 j1
    wc[2] = wt[64:192, 128:256]  # b1 j0
    wc[3] = wt[192:320, 128:256]  # b1 j1
    wc[4] = wt[192:320, 256:384]  # b2 j0
    wc[5] = wt[320:448, 256:384]  # b2 j1
    wc[6] = wt[320:448, 384:512]  # b3 j0
    return wc


def _band_xblocks(xbc):
    bl = np.zeros((4, P, S), np.float32)
    bl[0][0:64] = xbc[0:64]
    bl[0][64:128] = xbc[448:512]
    bl[1] = xbc[64:192]
    bl[2] = xbc[192:320]
    bl[3] = xbc[320:448]
    return bl


def _build_nc_band():
    # Raw Bass (no TileContext); one semaphore wait per instruction.
    #
    # 16 bf16 matmuls (2 per (bc, output-chunk) group) accumulate into 8 PSUM
    # banks. PSUM->SBUF bf16 downcast copies alternate between DVE (even
    # groups) and ACT (odd groups; GpSimd has no PSUM port); a dummy ACT copy
    # right after the input issues preloads the activation table so the
    # 1.3us ACT_TABLE_LOAD doesn't land on the first real copy. Per-group
    # 128 KiB stores go back out on the sync queue, which is idle once its
    # input pieces are issued.
    bf = mybir.dt.bfloat16
    f32 = mybir.dt.float32
    n_warm = 2   # fat (512-col fp32) PE warm-up matmuls during the input wait

    nc = bass.Bass()
    slab_p = nc.declare_dram_parameter("slab_p", [P, P_COLS], bf, isOutput=False)
    slab_q = nc.declare_dram_parameter("slab_q", [P, Q_COLS], bf, isOutput=False)
    out = nc.declare_dram_parameter("out", [BC_PER_CORE, P, 4 * S], bf, isOutput=True)

    with ExitStack() as ctx:
        p_sb = ctx.enter_context(nc.sbuf_tensor("p_sb", [P, P_COLS], bf))
        q_sb = ctx.enter_context(nc.sbuf_tensor("q_sb", [P, Q_COLS], bf))
        warm_sb = ctx.enter_context(nc.sbuf_tensor("warm_sb", [P, 5 * P + 32], f32))
        o_sb = [
            ctx.enter_context(nc.sbuf_tensor(f"o_sb{j}", [P, 4 * S], bf))
            for j in range(BC_PER_CORE)
        ]
        accs = [
            ctx.enter_context(nc.psum_tensor(f"acc{g}", [P, S], f32))
            for g in range(BC_PER_CORE * 4)
        ]
        s_p = ctx.enter_context(nc.semaphore("s_p"))
        s_q = ctx.enter_context(nc.semaphore("s_q"))
        s_warm = ctx.enter_context(nc.semaphore("s_warm"))
        s_pe = ctx.enter_context(nc.semaphore("s_pe"))
        s_cv = ctx.enter_context(nc.semaphore("s_cv"))  # DVE copies (even g)
        s_cp = ctx.enter_context(nc.semaphore("s_cp"))  # ACT copies (odd g)
        s_out = ctx.enter_context(nc.semaphore("s_out"))
        block = ctx.enter_context(nc.Block())

        def sb_of(slab):
            return p_sb if slab == "p" else q_sb

        @block.sync
        def _(sync):
            for lo, hi in P_PIECES:
                sync.dma_start(p_sb[:, lo:hi], slab_p[:, lo:hi]).then_inc(s_p, 16)
            # Per-group 128 KiB stores, issued as each group's copy lands.
            for g in range(BC_PER_CORE * 4):
                bc, b = divmod(g, 4)
                if g % 2 == 0:
                    sync.wait_ge(s_cv, g // 2 + 1)
                else:
                    sync.wait_ge(s_cp, g // 2 + 1)
                sync.dma_start(
                    out[bc, :, bass.ts(b, S)], o_sb[bc][:, bass.ts(b, S)]
                ).then_inc(s_out, 16)
            sync.wait_ge(s_out, BC_PER_CORE * 4 * 16)

        @block.tensor
        def _(tensor):
            # Warm-up matmuls on a zeroed scratch tile keep the PE busy while
            # inputs stream in, so HAM grants full clock as real work starts.
            tensor.wait_ge(s_warm, 1)
            for _ in range(n_warm):
                nc.tensor.matmul(
                    accs[-1][:], warm_sb[:, :P], warm_sb[:, P: 5 * P],
                    start=True, stop=True,
                )
            for bc in range(BC_PER_CORE):
                pa = qa = 0
                for b in range(4):
                    np_, nq = GROUP_THRESH[(bc, b)]
                    if np_ > pa:
                        tensor.wait_ge(s_p, np_ * 16)
                        pa = np_
                    if nq > qa:
                        tensor.wait_ge(s_q, nq * 16)
                        qa = nq
                    for j, (w, (p0, p1), blk) in enumerate(BAND_PLAN[b]):
                        wsl, wcol = WCOL_AT[w]
                        xsl, xcol = XBLK_AT[(bc, blk)]
                        mm = nc.tensor.matmul(
                            accs[4 * bc + b][:],
                            sb_of(wsl)[p0:p1, wcol: wcol + P],
                            sb_of(xsl)[p0:p1, xcol: xcol + S],
                            start=(j == 0),
                            stop=(j == 1),
                        )
                        if j == 1:
                            mm.then_inc(s_pe, 1)

        @block.vector
        def _(vector):
            nc.vector.memset(warm_sb[:], 0.0).then_inc(s_warm, 1)
            for g in range(0, BC_PER_CORE * 4, 2):
                bc, b = divmod(g, 4)
                vector.wait_ge(s_pe, g + 1)
                nc.vector.tensor_copy(
                    o_sb[bc][:, bass.ts(b, S)], accs[g][:]
                ).then_inc(s_cv, 1)

        @block.scalar
        def _(scalar):
            for lo, hi in Q_PIECES:
                scalar.dma_start(q_sb[:, lo:hi], slab_q[:, lo:hi]).then_inc(s_q, 16)
            # Dummy 1-col copy: pulls the ACT function table load off the
            # first real PSUM drain.
            nc.scalar.copy(o_sb[0][:, 0:1], warm_sb[:, 0:1])
            for g in range(1, BC_PER_CORE * 4, 2):
                bc, b = divmod(g, 4)
                scalar.wait_ge(s_pe, g + 1)
                nc.scalar.copy(
                    o_sb[bc][:, bass.ts(b, S)], accs[g][:]
                ).then_inc(s_cp, 1)

    return nc


def _run_band(x16, wt, trace):
    bf16 = _bf16()
    wc = _band_wcols(wt).astype(bf16)
    in_maps = []
    for core in range(N_CORES):
        bl0 = _band_xblocks(x16[BC_PER_CORE * core]).astype(bf16)
        bl1 = _band_xblocks(x16[BC_PER_CORE * core + 1]).astype(bf16)
        slab_p = np.concatenate(
            [wc[0], wc[1], bl0[0], wc[2], wc[3], bl0[2], wc[6], bl1[2]], axis=1
        )
        slab_q = np.concatenate(
            [bl0[1], wc[4], wc[5], bl0[3], bl1[0], bl1[1], bl1[3]], axis=1
        )
        in_maps.append(
            {
                "slab_p": np.ascontiguousarray(slab_p),
                "slab_q": np.ascontiguousarray(slab_q),
            }
        )
    nc = _build_nc_band()
    res = run_bass_kernel_spmd(nc, in_maps, core_ids=list(range(N_CORES)), trace=trace)
    # out[bc, p, 512*b + a] -> q[bc_global, 128*b + p, a]
    q = np.concatenate(
        [
            np.asarray(res.results[k]["out"], dtype=np.float32)
            .reshape(BC_PER_CORE, P, 4, S)
            .transpose(0, 2, 1, 3)
            .reshape(BC_PER_CORE, S, S)
            for k in range(N_CORES)
        ],
        axis=0,
    )
    return q, res


# ---------------------------------------------------------------------------
# Dense fallback (exact, fp32r) -- the original data layout: 16 matmuls/core.
# ---------------------------------------------------------------------------

def _build_nc_dense():
    mmdt = mybir.dt.float32r
    kc = S // P  # 4 contraction chunks
    oc = S // P  # 4 output-row chunks
    f32 = mybir.dt.float32
    n_warm = 3

    nc = bass.Bass()
    wx = nc.declare_dram_parameter("wx", [kc, P, 2 * S], mmdt, isOutput=False)
    x1d = nc.declare_dram_parameter("x1", [kc, P, S], mmdt, isOutput=False)
    out = nc.declare_dram_parameter("out", [BC_PER_CORE, S, S], f32, isOutput=True)

    with ExitStack() as ctx:
        wx_sb = [
            ctx.enter_context(nc.sbuf_tensor(f"wx_sb{k}", [P, 2 * S], mmdt))
            for k in range(kc)
        ]
        x1_sb = ctx.enter_context(nc.sbuf_tensor("x1_sb", [P, 4 * S], mmdt))
        warm_sb = ctx.enter_context(nc.sbuf_tensor("warm_sb", [P, 3 * P + 32], f32))
        o_sb = [
            ctx.enter_context(nc.sbuf_tensor(f"o_sb{j}", [P, 4 * S], f32))
            for j in range(2)
        ]
        accs = [
            ctx.enter_context(nc.psum_tensor(f"acc{g}", [P, S], f32))
            for g in range(BC_PER_CORE * oc)
        ]
        s_wx = [ctx.enter_context(nc.semaphore(f"s_wx{k}")) for k in range(kc)]
        s_x1 = [ctx.enter_context(nc.semaphore(f"s_x1{k}")) for k in range(kc)]
        s_warm = ctx.enter_context(nc.semaphore("s_warm"))
        s_pe = ctx.enter_context(nc.semaphore("s_pe"))
        s_dve = ctx.enter_context(nc.semaphore("s_dve"))
        s_out = ctx.enter_context(nc.semaphore("s_out"))
        block = ctx.enter_context(nc.Block())

        @block.sync
        def _(sync):
            for k in range(kc):
                sync.dma_start(wx_sb[k][:], wx[k]).then_inc(s_wx[k], 16)
            for k in range(kc):
                sync.dma_start(x1_sb[:, bass.ts(k, S)], x1d[k]).then_inc(s_x1[k], 16)
            sync.wait_ge(s_out, BC_PER_CORE * oc * 16)

        @block.tensor
        def _(tensor):
            tensor.wait_ge(s_warm, 1)
            for _ in range(n_warm):
                nc.tensor.matmul(
                    accs[-1][:, : 2 * P], warm_sb[:, :P], warm_sb[:, P: 3 * P],
                    start=True, stop=True,
                )
            for k in range(kc):
                tensor.wait_ge(s_wx[k], 16)
                for o in range(oc):
                    mm = nc.tensor.matmul(
                        accs[o][:],
                        wx_sb[k][:, bass.ts(o, P)],
                        wx_sb[k][:, S: 2 * S],
                        start=(k == 0),
                        stop=(k == kc - 1),
                    )
                    if k == kc - 1:
                        mm.then_inc(s_pe, 1)
            for k in range(kc):
                tensor.wait_ge(s_x1[k], 16)
                for o in range(oc):
                    mm = nc.tensor.matmul(
                        accs[oc + o][:],
                        wx_sb[k][:, bass.ts(o, P)],
                        x1_sb[:, bass.ts(k, S)],
                        start=(k == 0),
                        stop=(k == kc - 1),
                    )
                    if k == kc - 1:
                        mm.then_inc(s_pe, 1)

        @block.vector
        def _(vector):
            nc.vector.memset(warm_sb[:], 0.0).then_inc(s_warm, 1)
            for g in range(BC_PER_CORE * oc):
                bc, o = divmod(g, oc)
                vector.wait_ge(s_pe, g + 1)
                nc.vector.tensor_copy(
                    o_sb[bc][:, bass.ts(o, S)], accs[g][:]
                ).then_inc(s_dve, 1)

        @block.scalar
        def _(scalar):
            for g in range(BC_PER_CORE * oc):
                bc, o = divmod(g, oc)
                scalar.wait_ge(s_dve, g + 1)
                scalar.dma_start(
                    out[bc, bass.ts(o, P), :], o_sb[bc][:, bass.ts(o, S)]
                ).then_inc(s_out, 16)

    return nc


def _run_dense(x16, wt, trace):
    x16k = x16.reshape(BC_PER_CORE * N_CORES, S // P, P, S)
    wt4 = wt.reshape(S // P, P, S)
    in_maps = []
    for core in range(N_CORES):
        x0 = x16k[BC_PER_CORE * core]
        x1 = x16k[BC_PER_CORE * core + 1]
        wx = np.concatenate([wt4, x0], axis=2)
        in_maps.append(
            {
                "wx": np.ascontiguousarray(wx),
                "x1": np.ascontiguousarray(x1),
            }
        )
    nc = _build_nc_dense()
    res = run_bass_kernel_spmd(nc, in_maps, core_ids=list(range(N_CORES)), trace=trace)
    q = np.concatenate(
        [np.asarray(res.results[k]["out"], dtype=np.float32) for k in range(N_CORES)],
        axis=0,
    )
    return q, res


def kernel(x, twiddle_fft, twiddle_ifft, fourier_filter_br):
    global last_exec_time_ns, last_results, last_path
    x = np.asarray(x, dtype=np.float32)
    b, c, s_len, a = x.shape
    assert (b, c, s_len, a) == (8, 2, S, S)

    wt = _compose_wt(twiddle_fft, twiddle_ifft, fourier_filter_br)
    x16 = x.reshape(b * c, S, S)  # [bc, row, angle]
    trace = os.environ.get("BUTTERFLY_TRACE") == "1"

    use_band = (
        os.environ.get("BUTTERFLY_FORCE_DENSE") != "1"
        and _band_error(wt) < BAND_ERR_MAX
    )
    if use_band:
        q, res = _run_band(x16, wt, trace)
        last_path = "band"
    else:
        q, res = _run_dense(x16, wt, trace)
        last_path = "dense"
    last_exec_time_ns = res.exec_time_ns
    last_results = res

    # q[bc, o, a] = proj.T[o, bc*512 + a]; reference output is
    # proj.T.reshape(b, c, s, a) — a pure reinterpret of the (512, 8192) buffer.
    out = q.transpose(1, 0, 2).reshape(S, b * c * a).reshape(b, c, s_len, a)
    return np.ascontiguousarray(out).astype(np.float32)


# revision 14
# speedup vs baseline: 1.0259x; 1.0259x over previous
"""Trainium2 Bass kernel for nn_ButterflyFilter.

The reference applies, per length-512 row (flattened b*c*angles):
  zero-pad to 1024 -> 10-stage butterfly "FFT" (stage order decreasing)
  -> elementwise filter (bit-reversed order) -> 10-stage butterfly
  "IFFT" (stage order increasing) -> real part of first 512 entries.

Every step is linear in x, so the whole chain is one complex 1024x1024
operator A determined by (twiddle_fft, twiddle_ifft, fourier_filter_br).
Since x is real with support on [:512] and only Re(y)[:512] is kept, the
effective map is the real 512x512 matrix W = Re(A)[:512, :512]:

    proj_row = W @ x_row

For the actual FBP ramp-filter parameters, W is a circular-convolution
(Toeplitz) matrix whose kernel decays as 1/k^2: truncating it to a band
of half-width 64 costs ~1.7e-4 relative error (gate is 2e-2). Each
128-row output chunk b then only needs contraction rows
[128b-64, 128b+192), i.e. TWO 128-row windows from the offset-64
partitioning of the input rows -- 2 matmuls per chunk instead of 4, in
bf16 (~2.6e-3 total err). The device work per core is 2 (b,c) tiles x
4 output chunks x 2 matmuls. If the runtime-composed W turns out not to
be banded (e.g. random twiddles), we fall back to the exact dense fp32r
path automatically.
"""

import os
import sys
import types
from contextlib import ExitStack

import numpy as np

import concourse.bass as bass
import concourse.mybir as mybir
from concourse.bass_utils import run_bass_kernel_spmd


def _ensure_axon_hooks():
    # concourse.bass_utils imports antenv.axon_hooks on the trace path; some
    # images lack that module. Provide a no-op holder so a BASS_TRACE env set
    # by the caller can't crash the run.
    try:
        import antenv.axon_hooks  # noqa: F401
    except Exception:
        m = types.ModuleType("antenv.axon_hooks")
        m._h = None
        m.set_axon_ntff_profile_hook = lambda h: setattr(m, "_h", h)
        m.get_axon_ntff_profile_hook = lambda: m._h
        sys.modules["antenv.axon_hooks"] = m


_ensure_axon_hooks()

N_CORES = 8
S = 512          # input/output row length
NF = 1024        # padded length
P = 128          # SBUF partitions
BC_PER_CORE = 2  # 16 (b,c) tiles / 8 cores

BAND_ERR_MAX = 5e-3  # operator-level truncation error gate for band path

# ---------------------------------------------------------------------------
# Band scheme geometry.
#
# Input rows are partitioned at the offset-64 boundaries [0,64,192,320,448,
# 512]; chunks c0=[0,64) and c4=[448,512) share one 128-partition SBUF block
# (c0 in partitions 0:64, c4 in 64:128), giving 4 x-blocks per (b,c) tile.
# Output chunk b consumes x-blocks {b, b+1 mod wrap} via W column-tiles:
#   wcol0 = b0j0 (c0 rows, partitions 0:64) + b3j1 (c4 rows, partitions
#   64:128); wcol1..6 = the full-128-row tiles.
# Per output chunk b: [(wcol, partition range, x block)] x 2 matmuls.
BAND_PLAN = {
    0: [(0, (0, 64), 0), (1, (0, 128), 1)],
    1: [(2, (0, 128), 1), (3, (0, 128), 2)],
    2: [(4, (0, 128), 2), (5, (0, 128), 3)],
    3: [(6, (0, 128), 3), (0, (64, 128), 0)],
}

# The input is streamed over BOTH HWDGE queues in parallel (SP=sync and
# Activation=scalar engines each own one hardware queue). DRAM slabs are
# (128, cols) bf16, partition-major, split into contiguous column pieces;
# each piece's completion bumps that queue's semaphore by 16, and in-queue
# completion order lets one cumulative threshold per queue gate each matmul
# group.
#
# slab_p (sync queue), column layout and pieces:
#   P1 [0,768):     wcol0 | wcol1 | x0 blk0          -> with Q1: bc0 b0
#   P2 [768,1536):  wcol2 | wcol3 | x0 blk2          -> bc0 b1 (+Q1)
#   P3 [1536,1664): wcol6                            -> bc0 b3 (+Q2)
#   P4 [1664,2176): x1 blk2                          -> bc1 b1
# slab_q (scalar queue):
#   Q1 [0,512):     x0 blk1                          -> bc0 b0/b1
#   Q2 [512,1280):  wcol4 | wcol5 | x0 blk3          -> bc0 b2/b3
#   Q3 [1280,2304): x1 blk0 | x1 blk1               -> bc1 b0
#   Q4 [2304,2816): x1 blk3                          -> bc1 b2/b3
P_COLS = 2176
Q_COLS = 2816
P_PIECES = [(0, 768), (768, 1536), (1536, 1664), (1664, 2176)]
Q_PIECES = [(0, 512), (512, 1280), (1280, 2304), (2304, 2816)]
# Per (bc, b) group: input pieces it depends on. Pieces get their OWN
# semaphores: queue pieces are striped across physical DMA engines, so piece
# completions are NOT ordered and a cumulative per-queue count would let a
# later piece satisfy an earlier piece's wait (measured: whole-core garbage
# on a random subset of cores). The PE waits each piece's semaphore the
# first time a group needs it.
GROUP_PIECES = {
    (0, 0): [("p", 0), ("q", 0)],
    (0, 1): [("p", 1), ("q", 0)],
    (0, 2): [("p", 1), ("q", 1)],
    (0, 3): [("p", 2), ("q", 1), ("p", 0)],
    (1, 0): [("q", 2)],
    (1, 1): [("p", 3), ("q", 2)],
    (1, 2): [("p", 3), ("q", 3)],
    (1, 3): [("q", 3), ("q", 2)],
}
# SBUF addresses: wcol index -> (slab, col); (bc, blk) -> (slab, col).
WCOL_AT = {0: ("p", 0), 1: ("p", 128), 2: ("p", 768), 3: ("p", 896),
           4: ("q", 512), 5: ("q", 640), 6: ("p", 1536)}
XBLK_AT = {(0, 0): ("p", 256), (0, 1): ("q", 0), (0, 2): ("p", 1024),
           (0, 3): ("q", 768), (1, 0): ("q", 1280), (1, 1): ("q", 1792),
           (1, 2): ("p", 1664), (1, 3): ("q", 2304)}

# Exposed for the test harness: exec time of the last device run (ns), if
# profiling was enabled via BUTTERFLY_TRACE=1.
last_exec_time_ns = None
last_results = None
last_path = None  # "band" or "dense", for the harness/debugging


def _bf16():
    return mybir.dt.np(mybir.dt.bfloat16)


def _butterfly_np(tw, x, increasing):
    # Mirrors the reference butterfly exactly, in numpy (any dtype).
    B, n = x.shape
    m = tw.shape[0]
    order = range(m) if increasing else range(m - 1, -1, -1)
    for idx in order:
        s = 1 << idx
        t = tw[idx].reshape(n // (2 * s), s, 2, 2)
        xr = x.reshape(B, n // (2 * s), 2, s)
        x = np.einsum('gjik,bgkj->bgij', t, xr).reshape(B, n)
    return x


def _compose_wt(twiddle_fft, twiddle_ifft, fourier_filter_br):
    """Fold twiddles+filter into the lhsT operand Wt[i_in, o_out] (512x512 f32)."""
    tw_fft = np.asarray(twiddle_fft, dtype=np.float64)
    tw_ifft = np.asarray(twiddle_ifft, dtype=np.float64)
    filt = np.asarray(fourier_filter_br, dtype=np.float64)
    tf = tw_fft[0, ..., 0] + 1j * tw_fft[0, ..., 1]
    ti = tw_ifft[0, ..., 0] + 1j * tw_ifft[0, ..., 1]
    X = np.eye(NF, dtype=np.complex128)      # row j = e_j
    X = _butterfly_np(tf, X, increasing=False)
    X = X * filt[None, :]
    X = _butterfly_np(ti, X, increasing=True)
    # X = chain(I) = A^T, so X[i, o] = A[o, i]; W[o, i] = Re(A[o, i]).
    # lhsT for out = lhsT.T @ rhs must be Wt[i, o] = W[o, i] = Re(X[i, o]).
    return np.ascontiguousarray(np.real(X[:S, :S]).astype(np.float32))


def _band_error(wt):
    """Relative Frobenius mass of W outside the 2-window band cover."""
    cov = np.zeros((S, S), bool)  # indexed [i, o] like wt
    for b in range(4):
        lo, hi = max(0, 128 * b - 64), min(S, 128 * b + 192)
        cov[lo:hi, 128 * b:128 * b + 128] = True
    tot = float(np.square(wt).sum())
    off = float(np.square(wt[~cov]).sum())
    return (off / tot) ** 0.5 if tot > 0 else 0.0


def _band_wcols(wt):
    wc = np.zeros((7, P, P), np.float32)
    wc[0][0:64] = wt[0:64, 0:128]          # b0 j0 (chunk c0)
    wc[0][64:128] = wt[448:512, 384:512]   # b3 j1 (chunk c4)
    wc[1] = wt[64:192, 0:128]    # b0 j1
    wc[2] = wt[64:192, 128:256]  # b1 j0
    wc[3] = wt[192:320, 128:256]  # b1 j1
    wc[4] = wt[192:320, 256:384]  # b2 j0
    wc[5] = wt[320:448, 256:384]  # b2 j1
    wc[6] = wt[320:448, 384:512]  # b3 j0
    return wc


def _band_xblocks(xbc):
    bl = np.zeros((4, P, S), np.float32)
    bl[0][0:64] = xbc[0:64]
    bl[0][64:128] = xbc[448:512]
    bl[1] = xbc[64:192]
    bl[2] = xbc[192:320]
    bl[3] = xbc[320:448]
    return bl


def _build_nc_band():
    # Raw Bass (no TileContext); one semaphore wait per instruction.
    #
    # 16 bf16 matmuls (2 per (bc, output-chunk) group) accumulate into 8 PSUM
    # banks. PSUM->SBUF bf16 downcast copies alternate between DVE (even
    # groups) and ACT (odd groups; GpSimd has no PSUM port); a dummy ACT copy
    # right after the input issues preloads the activation table so the
    # 1.3us ACT_TABLE_LOAD doesn't land on the first real copy. Per-group
    # 128 KiB stores go back out on the sync queue, which is idle once its
    # input pieces are issued.
    bf = mybir.dt.bfloat16
    f32 = mybir.dt.float32
    n_warm = 2   # fat (512-col fp32) PE warm-up matmuls during the input wait

    nc = bass.Bass()
    slab_p = nc.declare_dram_parameter("slab_p", [P, P_COLS], bf, isOutput=False)
    slab_q = nc.declare_dram_parameter("slab_q", [P, Q_COLS], bf, isOutput=False)
    out = nc.declare_dram_parameter("out", [BC_PER_CORE, P, 4 * S], bf, isOutput=True)

    with ExitStack() as ctx:
        p_sb = ctx.enter_context(nc.sbuf_tensor("p_sb", [P, P_COLS], bf))
        q_sb = ctx.enter_context(nc.sbuf_tensor("q_sb", [P, Q_COLS], bf))
        warm_sb = ctx.enter_context(nc.sbuf_tensor("warm_sb", [P, 5 * P + 32], f32))
        dummy_sb = ctx.enter_context(nc.sbuf_tensor("dummy_sb", [P, 2], f32))
        o_sb = [
            ctx.enter_context(nc.sbuf_tensor(f"o_sb{j}", [P, 4 * S], bf))
            for j in range(BC_PER_CORE)
        ]
        accs = [
            ctx.enter_context(nc.psum_tensor(f"acc{g}", [P, S], f32))
            for g in range(BC_PER_CORE * 4)
        ]
        s_p = [
            ctx.enter_context(nc.semaphore(f"s_p{i}")) for i in range(len(P_PIECES))
        ]
        s_q = [
            ctx.enter_context(nc.semaphore(f"s_q{i}")) for i in range(len(Q_PIECES))
        ]
        s_warm = ctx.enter_context(nc.semaphore("s_warm"))
        s_pe = ctx.enter_context(nc.semaphore("s_pe"))
        s_cv = ctx.enter_context(nc.semaphore("s_cv"))  # DVE copies (even g)
        s_cp = ctx.enter_context(nc.semaphore("s_cp"))  # ACT copies (odd g)
        s_out = ctx.enter_context(nc.semaphore("s_out"))
        block = ctx.enter_context(nc.Block())

        def sb_of(slab):
            return p_sb if slab == "p" else q_sb

        @block.sync
        def _(sync):
            for i, (lo, hi) in enumerate(P_PIECES):
                sync.dma_start(p_sb[:, lo:hi], slab_p[:, lo:hi]).then_inc(s_p[i], 16)
            # Per-group 128 KiB stores, issued as each group's copy lands.
            for g in range(BC_PER_CORE * 4):
                bc, b = divmod(g, 4)
                if g % 2 == 0:
                    sync.wait_ge(s_cv, g // 2 + 1)
                else:
                    sync.wait_ge(s_cp, g // 2 + 1)
                sync.dma_start(
                    out[bc, :, bass.ts(b, S)], o_sb[bc][:, bass.ts(b, S)]
                ).then_inc(s_out, 16)
            sync.wait_ge(s_out, BC_PER_CORE * 4 * 16)

        @block.tensor
        def _(tensor):
            # Warm-up matmuls on a zeroed scratch tile keep the PE busy while
            # inputs stream in, so HAM grants full clock as real work starts.
            tensor.wait_ge(s_warm, 1)
            for _ in range(n_warm):
                nc.tensor.matmul(
                    accs[-1][:], warm_sb[:, :P], warm_sb[:, P: 5 * P],
                    start=True, stop=True,
                )
            waited = set()
            for bc in range(BC_PER_CORE):
                for b in range(4):
                    for piece in GROUP_PIECES[(bc, b)]:
                        if piece in waited:
                            continue
                        waited.add(piece)
                        sl, i = piece
                        tensor.wait_ge((s_p if sl == "p" else s_q)[i], 16)
                    for j, (w, (p0, p1), blk) in enumerate(BAND_PLAN[b]):
                        wsl, wcol = WCOL_AT[w]
                        xsl, xcol = XBLK_AT[(bc, blk)]
                        mm = nc.tensor.matmul(
                            accs[4 * bc + b][:],
                            sb_of(wsl)[p0:p1, wcol: wcol + P],
                            sb_of(xsl)[p0:p1, xcol: xcol + S],
                            start=(j == 0),
                            stop=(j == 1),
                        )
                        if j == 1:
                            mm.then_inc(s_pe, 1)

        @block.vector
        def _(vector):
            nc.vector.memset(warm_sb[:], 0.0).then_inc(s_warm, 1)
            for g in range(0, BC_PER_CORE * 4, 2):
                bc, b = divmod(g, 4)
                vector.wait_ge(s_pe, g + 1)
                nc.vector.tensor_copy(
                    o_sb[bc][:, bass.ts(b, S)], accs[g][:]
                ).then_inc(s_cv, 1)

        @block.scalar
        def _(scalar):
            for i, (lo, hi) in enumerate(Q_PIECES):
                scalar.dma_start(q_sb[:, lo:hi], slab_q[:, lo:hi]).then_inc(s_q[i], 16)
            # Dummy 1-col copy on a private scratch tile: pulls the ACT
            # function table load off the first real PSUM drain.
            nc.scalar.copy(dummy_sb[:, 0:1], dummy_sb[:, 1:2])
            for g in range(1, BC_PER_CORE * 4, 2):
                bc, b = divmod(g, 4)
                scalar.wait_ge(s_pe, g + 1)
                nc.scalar.copy(
                    o_sb[bc][:, bass.ts(b, S)], accs[g][:]
                ).then_inc(s_cp, 1)

    return nc


def _run_band(x16, wt, trace):
    bf16 = _bf16()
    wc = _band_wcols(wt).astype(bf16)
    in_maps = []
    for core in range(N_CORES):
        bl0 = _band_xblocks(x16[BC_PER_CORE * core]).astype(bf16)
        bl1 = _band_xblocks(x16[BC_PER_CORE * core + 1]).astype(bf16)
        slab_p = np.concatenate(
            [wc[0], wc[1], bl0[0], wc[2], wc[3], bl0[2], wc[6], bl1[2]], axis=1
        )
        slab_q = np.concatenate(
            [bl0[1], wc[4], wc[5], bl0[3], bl1[0], bl1[1], bl1[3]], axis=1
        )
        in_maps.append(
            {
                "slab_p": np.ascontiguousarray(slab_p),
                "slab_q": np.ascontiguousarray(slab_q),
            }
        )
    nc = _build_nc_band()
    res = run_bass_kernel_spmd(nc, in_maps, core_ids=list(range(N_CORES)), trace=trace)
    # out[bc, p, 512*b + a] -> q[bc_global, 128*b + p, a]
    q = np.concatenate(
        [
            np.asarray(res.results[k]["out"], dtype=np.float32)
            .reshape(BC_PER_CORE, P, 4, S)
            .transpose(0, 2, 1, 3)
            .reshape(BC_PER_CORE, S, S)
            for k in range(N_CORES)
        ],
        axis=0,
    )
    return q, res


# ---------------------------------------------------------------------------
# Dense fallback (exact, fp32r) -- the original data layout: 16 matmuls/core.
# ---------------------------------------------------------------------------

def _build_nc_dense():
    mmdt = mybir.dt.float32r
    kc = S // P  # 4 contraction chunks
    oc = S // P  # 4 output-row chunks
    f32 = mybir.dt.float32
    n_warm = 3

    nc = bass.Bass()
    wx = nc.declare_dram_parameter("wx", [kc, P, 2 * S], mmdt, isOutput=False)
    x1d = nc.declare_dram_parameter("x1", [kc, P, S], mmdt, isOutput=False)
    out = nc.declare_dram_parameter("out", [BC_PER_CORE, S, S], f32, isOutput=True)

    with ExitStack() as ctx:
        wx_sb = [
            ctx.enter_context(nc.sbuf_tensor(f"wx_sb{k}", [P, 2 * S], mmdt))
            for k in range(kc)
        ]
        x1_sb = ctx.enter_context(nc.sbuf_tensor("x1_sb", [P, 4 * S], mmdt))
        warm_sb = ctx.enter_context(nc.sbuf_tensor("warm_sb", [P, 3 * P + 32], f32))
        o_sb = [
            ctx.enter_context(nc.sbuf_tensor(f"o_sb{j}", [P, 4 * S], f32))
            for j in range(2)
        ]
        accs = [
            ctx.enter_context(nc.psum_tensor(f"acc{g}", [P, S], f32))
            for g in range(BC_PER_CORE * oc)
        ]
        s_wx = [ctx.enter_context(nc.semaphore(f"s_wx{k}")) for k in range(kc)]
        s_x1 = [ctx.enter_context(nc.semaphore(f"s_x1{k}")) for k in range(kc)]
        s_warm = ctx.enter_context(nc.semaphore("s_warm"))
        s_pe = ctx.enter_context(nc.semaphore("s_pe"))
        s_dve = ctx.enter_context(nc.semaphore("s_dve"))
        s_out = ctx.enter_context(nc.semaphore("s_out"))
        block = ctx.enter_context(nc.Block())

        @block.sync
        def _(sync):
            for k in range(kc):
                sync.dma_start(wx_sb[k][:], wx[k]).then_inc(s_wx[k], 16)
            for k in range(kc):
                sync.dma_start(x1_sb[:, bass.ts(k, S)], x1d[k]).then_inc(s_x1[k], 16)
            sync.wait_ge(s_out, BC_PER_CORE * oc * 16)

        @block.tensor
        def _(tensor):
            tensor.wait_ge(s_warm, 1)
            for _ in range(n_warm):
                nc.tensor.matmul(
                    accs[-1][:, : 2 * P], warm_sb[:, :P], warm_sb[:, P: 3 * P],
                    start=True, stop=True,
                )
            for k in range(kc):
                tensor.wait_ge(s_wx[k], 16)
                for o in range(oc):
                    mm = nc.tensor.matmul(
                        accs[o][:],
                        wx_sb[k][:, bass.ts(o, P)],
                        wx_sb[k][:, S: 2 * S],
                        start=(k == 0),
                        stop=(k == kc - 1),
                    )
                    if k == kc - 1:
                        mm.then_inc(s_pe, 1)
            for k in range(kc):
                tensor.wait_ge(s_x1[k], 16)
                for o in range(oc):
                    mm = nc.tensor.matmul(
                        accs[oc + o][:],
                        wx_sb[k][:, bass.ts(o, P)],
                        x1_sb[:, bass.ts(k, S)],
                        start=(k == 0),
                        stop=(k == kc - 1),
                    )
                    if k == kc - 1:
                        mm.then_inc(s_pe, 1)

        @block.vector
        def _(vector):
            nc.vector.memset(warm_sb[:], 0.0).then_inc(s_warm, 1)
            for g in range(BC_PER_CORE * oc):
                bc, o = divmod(g, oc)
                vector.wait_ge(s_pe, g + 1)
                nc.vector.tensor_copy(
                    o_sb[bc][:, bass.ts(o, S)], accs[g][:]
                ).then_inc(s_dve, 1)

        @block.scalar
        def _(scalar):
            for g in range(BC_PER_CORE * oc):
                bc, o = divmod(g, oc)
                scalar.wait_ge(s_dve, g + 1)
                scalar.dma_start(
                    out[bc, bass.ts(o, P), :], o_sb[bc][:, bass.ts(o, S)]
                ).then_inc(s_out, 16)

    return nc


def _run_dense(x16, wt, trace):
    x16k = x16.reshape(BC_PER_CORE * N_CORES, S // P, P, S)
    wt4 = wt.reshape(S // P, P, S)
    in_maps = []
    for core in range(N_CORES):
        x0 = x16k[BC_PER_CORE * core]
        x1 = x16k[BC_PER_CORE * core + 1]
        wx = np.concatenate([wt4, x0], axis=2)
        in_maps.append(
            {
                "wx": np.ascontiguousarray(wx),
                "x1": np.ascontiguousarray(x1),
            }
        )
    nc = _build_nc_dense()
    res = run_bass_kernel_spmd(nc, in_maps, core_ids=list(range(N_CORES)), trace=trace)
    q = np.concatenate(
        [np.asarray(res.results[k]["out"], dtype=np.float32) for k in range(N_CORES)],
        axis=0,
    )
    return q, res


def kernel(x, twiddle_fft, twiddle_ifft, fourier_filter_br):
    global last_exec_time_ns, last_results, last_path
    x = np.asarray(x, dtype=np.float32)
    b, c, s_len, a = x.shape
    assert (b, c, s_len, a) == (8, 2, S, S)

    wt = _compose_wt(twiddle_fft, twiddle_ifft, fourier_filter_br)
    x16 = x.reshape(b * c, S, S)  # [bc, row, angle]
    trace = os.environ.get("BUTTERFLY_TRACE") == "1"

    use_band = (
        os.environ.get("BUTTERFLY_FORCE_DENSE") != "1"
        and _band_error(wt) < BAND_ERR_MAX
    )
    if use_band:
        q, res = _run_band(x16, wt, trace)
        last_path = "band"
    else:
        q, res = _run_dense(x16, wt, trace)
        last_path = "dense"
    last_exec_time_ns = res.exec_time_ns
    last_results = res

    # q[bc, o, a] = proj.T[o, bc*512 + a]; reference output is
    # proj.T.reshape(b, c, s, a) — a pure reinterpret of the (512, 8192) buffer.
    out = q.transpose(1, 0, 2).reshape(S, b * c * a).reshape(b, c, s_len, a)
    return np.ascontiguousarray(out).astype(np.float32)
